# revision 2
# baseline (speedup 1.0000x reference)
"""BSI quantized linear kernel for Trainium2 (8 NeuronCores, SPMD).

Computes out = round(x*100)/100 @ (round(W*100)/100).T + b for
x [4096, 4096] f32, W [4096, 4096] f32, b [4096] f32.

Sharding: W and b are sharded over out_features across the 8 cores
(tensor/column parallel); x is replicated. Each core computes its
[4096, 512] slice of the output; the host concatenates slices.

Math strategy: the quantized values round(100*v) are small integers
(|.| <= ~550 for x, <= ~11 for W), exactly representable in fp16.
The GEMM runs in fp16 on the PE at full rate, accumulating exact
integer dot products in fp32 PSUM (|sum| << 2^24), then the result is
scaled by 1e-4 and bias is added. Rounding uses the fp32 magic-number
trick (+/- 1.5*2^23) which implements round-half-to-even, matching
jnp.round bit-for-bit on the integer grid.

Per-core pipeline (B-row stripes of 128):
  DMA x stripe (f32, natural layout)
  ACT:  t = 100*x + MAGIC            (f32, in place)
  POOL: q = t - MAGIC -> fp16        (integer-valued fp16)
  PE:   transpose 128x128 q blocks -> PSUM (d on partitions)
  DVE:  copy PSUM -> SBUF xT tiles
  PE:   32-step K accumulation matmul vs resident quantized W^T
  ACT:  out_sbuf = 1e-4 * psum
  DVE:  out_sbuf += bias (broadcast)
  DMA out stripe
"""

import numpy as np

_B, _D, _DOUT = 4096, 4096, 4096
_NCORES = 8
_OPER = _DOUT // _NCORES  # 512
_MAGIC = 12582912.0  # 1.5 * 2**23
_P = 128

_nc_cache = {}


def _build(B, D, OPER):
    import concourse.mybir as mybir
    import concourse.tile as tile
    from concourse import bacc
    from concourse.masks import make_identity

    f32 = mybir.dt.float32
    f16 = mybir.dt.float16
    Copy = mybir.ActivationFunctionType.Copy
    P = _P
    KT = D // P
    BT = B // P
    OT = OPER // P
    KG = 8  # transposed 128x128 fp16 blocks per PSUM bank
    NG = KT // KG

    nc = bacc.Bacc("TRN2", target_bir_lowering=False, debug=False,
                   num_devices=_NCORES)
    x_d = nc.dram_tensor("x", [B, D], f32, kind="ExternalInput").ap()
    w_d = nc.dram_tensor("w", [OPER, D], f32, kind="ExternalInput").ap()
    b_d = nc.dram_tensor("b", [OPER], f32, kind="ExternalInput").ap()
    o_d = nc.dram_tensor("out", [B, OPER], f32, kind="ExternalOutput").ap()

    with tile.TileContext(nc) as tc:
        with (
            tc.tile_pool(name="const", bufs=1) as cpool,
            tc.tile_pool(name="wq", bufs=1) as wpool,
            tc.tile_pool(name="stage", bufs=3) as spool,
            tc.tile_pool(name="q16", bufs=3) as qpool,
            tc.tile_pool(name="xT", bufs=3) as xtpool,
            tc.tile_pool(name="tps", bufs=3, space="PSUM") as tppool,
            tc.tile_pool(name="mmps", bufs=2, space="PSUM") as mmpool,
            tc.tile_pool(name="osb", bufs=3) as opool,
        ):
            ident = cpool.tile([P, P], f16)
            make_identity(nc, ident)
            bias_bc = cpool.tile([P, OPER], f32)
            nc.sync.dma_start(bias_bc, b_d[None, :].to_broadcast((P, OPER)))

            # Quantized, transposed W slice, SBUF-resident: [128, KT, OPER] fp16
            wT = wpool.tile([P, KT, OPER], f16)

            def load_quant_transpose(src_rows, dst_cols_fn):
                """DMA 128 rows x D f32, quantize to integer fp16, PE-transpose
                all KT 128x128 blocks, landing them via dst_cols_fn(g) slices."""
                st = spool.tile([P, D], f32, tag="stage")
                nc.sync.dma_start(st, src_rows)
                # t = fl32(100*x): single rounding, matches the reference's
                # f32 multiply exactly (bias=0 makes the ACT FMA a plain mult)
                nc.scalar.activation(st, st, Copy, bias=0.0, scale=100.0)
                q = qpool.tile([P, D], f16, tag="q16")
                # round-half-even to integer via the fp32 magic constant:
                # stage0 adds 1.5*2^23 (rounds to integer), stage1 subtracts it
                nc.gpsimd.tensor_scalar(q, st, _MAGIC, -_MAGIC,
                                        mybir.AluOpType.add,
                                        mybir.AluOpType.add)
                for g in range(NG):
                    tp = tppool.tile([P, KG, P], f16, tag="tps")
                    for j in range(KG):
                        kt = g * KG + j
                        nc.tensor.transpose(tp[:, j, :],
                                            q[:, kt * P:(kt + 1) * P], ident)
                    nc.vector.tensor_copy(dst_cols_fn(g), tp)

            # W preamble: quantize + transpose the whole W slice into SBUF
            for ot in range(OT):
                load_quant_transpose(
                    w_d[ot * P:(ot + 1) * P, :],
                    lambda g, ot=ot: wT[:, g * KG:(g + 1) * KG,
                                        ot * P:(ot + 1) * P],
                )

            # Main loop over B stripes
            for bt in range(BT):
                xT = xtpool.tile([P, KT, P], f16, tag="xT")
                load_quant_transpose(
                    x_d[bt * P:(bt + 1) * P, :],
                    lambda g, xT=xT: xT[:, g * KG:(g + 1) * KG, :],
                )
                ps = mmpool.tile([P, OPER], f32, tag="mmps")
                for kt in range(KT):
                    nc.tensor.matmul(ps, xT[:, kt, :], wT[:, kt, :],
                                     start=(kt == 0), stop=(kt == KT - 1))
                ob = opool.tile([P, OPER], f32, tag="osb")
                nc.scalar.activation(ob, ps, Copy, bias=0.0, scale=1e-4)
                nc.vector.tensor_add(ob, ob, bias_bc)
                nc.sync.dma_start(o_d[bt * P:(bt + 1) * P, :], ob)

    nc.compile()
    return nc


def _get_nc(B=_B, D=_D, OPER=_OPER):
    key = (B, D, OPER)
    if key not in _nc_cache:
        _nc_cache[key] = _build(B, D, OPER)
    return _nc_cache[key]


def _run(x, W, b, trace=False):
    from concourse.bass_utils import run_bass_kernel_spmd

    B, D = x.shape
    OALL = W.shape[0]
    OPER = OALL // _NCORES
    nc = _get_nc(B, D, OPER)
    in_maps = []
    for c in range(_NCORES):
        in_maps.append({
            "x": x,
            "w": np.ascontiguousarray(W[c * OPER:(c + 1) * OPER]),
            "b": np.ascontiguousarray(b[c * OPER:(c + 1) * OPER]),
        })
    res = run_bass_kernel_spmd(nc, in_maps, core_ids=list(range(_NCORES)),
                               trace=trace)
    out = np.concatenate([res.results[c]["out"] for c in range(_NCORES)],
                         axis=1)
    return out, res


def kernel(x=None, W=None, b=None):
    x = np.ascontiguousarray(np.asarray(x, dtype=np.float32))
    W = np.ascontiguousarray(np.asarray(W, dtype=np.float32))
    b = np.ascontiguousarray(np.asarray(b, dtype=np.float32))
    out, _ = _run(x, W, b, trace=False)
    return out


# revision 5
# speedup vs baseline: 1.9787x; 1.9787x over previous
"""BSI quantized linear kernel for Trainium2 (8 NeuronCores, SPMD).

Computes out = round(x*100)/100 @ (round(W*100)/100).T + b for
x [4096, 4096] f32, W [4096, 4096] f32, b [4096] f32.

Sharding: W and b are sharded over out_features across the 8 cores
(tensor/column parallel); x is replicated. Each core computes its
[4096, 512] slice of the output; the host concatenates slices.

Math strategy: the quantized values round(100*v) are small integers
(|.| <= ~550 for x, <= ~11 for W), exactly representable in fp16.
The GEMM runs in fp16 on the PE at full rate, accumulating exact
integer dot products in fp32 PSUM (|sum| << 2^24), then the result is
scaled by 1e-4 and bias is added. Rounding uses the fp32 magic-number
trick (+/- 1.5*2^23) which implements round-half-to-even, matching
jnp.round bit-for-bit on the integer grid.

Per-core pipeline (B-row stripes of 128):
  DMA x stripe (f32, natural layout)
  ACT:  t = 100*x + MAGIC            (f32, in place)
  POOL: q = t - MAGIC -> fp16        (integer-valued fp16)
  PE:   transpose 128x128 q blocks -> PSUM (d on partitions)
  DVE:  copy PSUM -> SBUF xT tiles
  PE:   32-step K accumulation matmul vs resident quantized W^T
  ACT:  out_sbuf = 1e-4 * psum
  DVE:  out_sbuf += bias (broadcast)
  DMA out stripe
"""

import numpy as np

_B, _D, _DOUT = 4096, 4096, 4096
_NCORES = 8
_OPER = _DOUT // _NCORES  # 512
_MAGIC = 12582912.0  # 1.5 * 2**23
_P = 128

_nc_cache = {}


def _build(B, D, OPER):
    import concourse.mybir as mybir
    import concourse.tile as tile
    from concourse import bacc
    from concourse.masks import make_identity

    f32 = mybir.dt.float32
    f16 = mybir.dt.float16
    Copy = mybir.ActivationFunctionType.Copy
    P = _P
    KT = D // P
    BT = B // P
    OT = OPER // P
    KG = 8  # transposed 128x128 fp16 blocks per PSUM bank
    NG = KT // KG

    nc = bacc.Bacc("TRN2", target_bir_lowering=False, debug=False,
                   num_devices=_NCORES)
    x_d = nc.dram_tensor("x", [B, D], f32, kind="ExternalInput").ap()
    w_d = nc.dram_tensor("w", [OPER, D], f32, kind="ExternalInput").ap()
    b_d = nc.dram_tensor("b", [OPER], f32, kind="ExternalInput").ap()
    o_d = nc.dram_tensor("out", [B, OPER], f32, kind="ExternalOutput").ap()

    with tile.TileContext(nc) as tc:
        with (
            tc.tile_pool(name="const", bufs=1) as cpool,
            tc.tile_pool(name="wq", bufs=1) as wpool,
            tc.tile_pool(name="stage", bufs=3) as spool,
            tc.tile_pool(name="q16", bufs=3) as qpool,
            tc.tile_pool(name="xT", bufs=3) as xtpool,
            tc.tile_pool(name="tps", bufs=3, space="PSUM") as tppool,
            tc.tile_pool(name="mmps", bufs=2, space="PSUM") as mmpool,
            tc.tile_pool(name="osb", bufs=3) as opool,
            tc.tile_pool(name="dram", bufs=1, space="DRAM") as dpool,
        ):
            ident = cpool.tile([P, P], f16)
            make_identity(nc, ident)
            bias_bc = cpool.tile([P, OPER], f32)
            nc.sync.dma_start(bias_bc, b_d[None, :].to_broadcast((P, OPER)))

            # Quantized, transposed W slice, SBUF-resident: [128, KT, OPER] fp16
            wT = wpool.tile([P, KT, OPER], f16)

            def load_quant_transpose(src_rows, dst_cols_fn):
                """DMA 128 rows x D f32, quantize to integer fp16, PE-transpose
                all KT 128x128 blocks, landing them via dst_cols_fn(g) slices."""
                st = spool.tile([P, D], f32, tag="stage")
                nc.sync.dma_start(st, src_rows)
                # t = fl32(fl32(100*x) + MAGIC): the DVE two-stage ALU rounds
                # to f32 between stages, so stage0 reproduces the reference's
                # f32 multiply and stage1's +1.5*2^23 rounds half-to-even to
                # the integer grid.
                nc.vector.tensor_scalar(st, st, 100.0, _MAGIC,
                                        mybir.AluOpType.mult,
                                        mybir.AluOpType.add)
                q = qpool.tile([P, D], f16, tag="q16")
                # subtract the magic constant back out (exact FMA, bias only)
                nc.scalar.activation(q, st, Copy, bias=-_MAGIC, scale=1.0)
                for g in range(NG):
                    tp = tppool.tile([P, KG, P], f16, tag="tps")
                    for j in range(KG):
                        kt = g * KG + j
                        nc.tensor.transpose(tp[:, j, :],
                                            q[:, kt * P:(kt + 1) * P], ident)
                    nc.vector.tensor_copy(dst_cols_fn(g), tp)

            # W preamble: quantize the W slice to integer fp16, round-trip it
            # through DRAM, and land the transposed [d, o] layout via the DMA
            # xbar (keeps PE/DVE free for the x pipeline)
            wscratch = dpool.tile([OPER, D], f16)
            for ot in range(OT):
                ws = spool.tile([P, D], f32, tag="stage")
                nc.sync.dma_start(ws, w_d[ot * P:(ot + 1) * P, :])
                nc.vector.tensor_scalar(ws, ws, 100.0, _MAGIC,
                                        mybir.AluOpType.mult,
                                        mybir.AluOpType.add)
                wq = qpool.tile([P, D], f16, tag="q16")
                nc.scalar.activation(wq, ws, Copy, bias=-_MAGIC, scale=1.0)
                nc.sync.dma_start(wscratch[ot * P:(ot + 1) * P, :], wq)
            for kt in range(KT):
                nc.sync.dma_start_transpose(wT[:, kt, :],
                                            wscratch[:, kt * P:(kt + 1) * P])

            # Main loop over B stripes
            for bt in range(BT):
                xT = xtpool.tile([P, KT, P], f16, tag="xT")
                load_quant_transpose(
                    x_d[bt * P:(bt + 1) * P, :],
                    lambda g, xT=xT: xT[:, g * KG:(g + 1) * KG, :],
                )
                ps = mmpool.tile([P, OPER], f32, tag="mmps")
                for kt in range(KT):
                    nc.tensor.matmul(ps, xT[:, kt, :], wT[:, kt, :],
                                     start=(kt == 0), stop=(kt == KT - 1))
                ob = opool.tile([P, OPER], f32, tag="osb")
                nc.scalar.activation(ob, ps, Copy, bias=0.0, scale=1e-4)
                nc.vector.tensor_add(ob, ob, bias_bc)
                nc.sync.dma_start(o_d[bt * P:(bt + 1) * P, :], ob)

    nc.compile()
    return nc


def _get_nc(B=_B, D=_D, OPER=_OPER):
    key = (B, D, OPER)
    if key not in _nc_cache:
        _nc_cache[key] = _build(B, D, OPER)
    return _nc_cache[key]


def _run(x, W, b, trace=False):
    from concourse.bass_utils import run_bass_kernel_spmd

    B, D = x.shape
    OALL = W.shape[0]
    OPER = OALL // _NCORES
    nc = _get_nc(B, D, OPER)
    in_maps = []
    for c in range(_NCORES):
        in_maps.append({
            "x": x,
            "w": np.ascontiguousarray(W[c * OPER:(c + 1) * OPER]),
            "b": np.ascontiguousarray(b[c * OPER:(c + 1) * OPER]),
        })
    res = run_bass_kernel_spmd(nc, in_maps, core_ids=list(range(_NCORES)),
                               trace=trace)
    out = np.concatenate([res.results[c]["out"] for c in range(_NCORES)],
                         axis=1)
    return out, res


def kernel(x=None, W=None, b=None):
    x = np.ascontiguousarray(np.asarray(x, dtype=np.float32))
    W = np.ascontiguousarray(np.asarray(W, dtype=np.float32))
    b = np.ascontiguousarray(np.asarray(b, dtype=np.float32))
    out, _ = _run(x, W, b, trace=False)
    return out
